# revision 20
# baseline (speedup 1.0000x reference)
"""ExpertGraphConv Trainium2 kernel (bf16 pipeline, native gelu, 6-stage skew).

Full inputs in, full output out. Shards batch dim (B=8) across 8 NeuronCores;
params replicated. Each core processes 2048 tokens x 8 experts = 16384 rows.

Math per (token t, expert i):
  adj = sigmoid(adjacency_logits); wa, wb = w_msg[:D], w_msg[D:]
  a[t,i] = x[t,i] . wa ; b[t,j] = x[t,j] . wb
  strength[t,i,j] = adj[i,j] * sigmoid(a[t,i]+b[t,j]+b_msg) * (i != j)
  msg[t,i] = sum_j strength[t,i,j] x[t,j]
  out = gelu(msg @ Wn^T + x @ Ws^T + bn + bs)

Per 128-row chunk (16 tokens x 8 experts), software-pipelined so every PE
instruction's cross-engine inputs are produced >= 1 full stage earlier:
  A(c):  DMA xin (fp32)
  B(c):  gpsimd cast -> xbf (bf16)
  C(c):  DVE a/b row-sums (fused mult+accum) -> ab6 rotating tile
  D(c):  PE transpose of ab6 ([128,35] bf16, groups at cols 0/32 so the
         transposed operands sit at legal base partitions) + DVE copies
  E(c):  PE K=3 outer matmul (scores for all 64 expert pairs) + ACT tanh
         + DVE strength stt + DVE bias prewrite into big PSUM
  F1(c): PE cmb: one bf16 matmul per d-chunk, rhs=[Sblk | I128] gives
         msg^T and x^T together in PSUM; ACT copy -> bf16 zt
  F2(c): 8 accumulating bf16 matmuls (start=False onto the prewritten
         bias) + native Gelu from PSUM + DMA out
Emission per iteration i: A(i), B(i-1), C(i-2), D(i-3), E(i-4), F1(i-5),
F2(i-6). The gelu/tanh pair lives in the 'gelu_and_others' ACT table set:
no table reloads in steady state.
"""

import math
from contextlib import ExitStack

import numpy as np

import concourse.bacc as bacc
import concourse.mybir as mybir
import concourse.tile as tile
from concourse import bass_utils
from concourse.masks import make_identity

F32 = mybir.dt.float32
F32R = mybir.dt.float32r
BF16 = mybir.dt.bfloat16
AF = mybir.ActivationFunctionType
OP = mybir.AluOpType

B, L, E, D = 8, 2048, 8, 512
N_CORES = 8
P = 128
ROWS_PER_CORE = (B // N_CORES) * L * E  # 16384
NSC = 8  # rotation depth of the static rotating buffers


def build_nc(n_rows=ROWS_PER_CORE, gelu_tanh_standin=False):
    assert n_rows % P == 0
    n_chunks = n_rows // P
    nd = D // P  # 4 d-chunks

    nc = bacc.Bacc(
        "TRN2", target_bir_lowering=False, debug=False, num_devices=N_CORES
    )

    x_dram = nc.dram_tensor("expert_features", [n_rows, D], F32, kind="ExternalInput").ap()
    wn_dram = nc.dram_tensor("W_neighbor", [D, D], F32, kind="ExternalInput").ap()
    bn_dram = nc.dram_tensor("b_neighbor", [1, D], F32, kind="ExternalInput").ap()
    ws_dram = nc.dram_tensor("W_self", [D, D], F32, kind="ExternalInput").ap()
    bs_dram = nc.dram_tensor("b_self", [1, D], F32, kind="ExternalInput").ap()
    wmsg_dram = nc.dram_tensor("w_msg", [1, 2 * D], F32, kind="ExternalInput").ap()
    bmsg_dram = nc.dram_tensor("b_msg", [1, 1], F32, kind="ExternalInput").ap()
    adj_dram = nc.dram_tensor("adjacency_logits", [E, E], F32, kind="ExternalInput").ap()
    out_dram = nc.dram_tensor("out", [n_rows, D], F32, kind="ExternalOutput").ap()

    with tile.TileContext(nc) as tc, ExitStack() as ctx:
        # ---- static SBUF tensors ----
        I128 = nc.alloc_sbuf_tensor("c_I128", [P, P], F32).ap()
        I128b = nc.alloc_sbuf_tensor("c_I128b", [P, P], BF16).ap()
        wa_rep = nc.alloc_sbuf_tensor("c_wa_rep", [P, D], F32).ap()
        wb_rep = nc.alloc_sbuf_tensor("c_wb_rep", [P, D], F32).ap()
        hadj_blk = nc.alloc_sbuf_tensor("c_hadj_blk", [P, P], BF16).ap()
        ones_row = nc.alloc_sbuf_tensor("c_ones_row", [1, P], F32).ap()
        ones_row_b = nc.alloc_sbuf_tensor("c_ones_row_b", [1, P], BF16).ap()
        bias_row_b = nc.alloc_sbuf_tensor("c_bias_row_b", [1, D], BF16).ap()
        qmask8 = nc.alloc_sbuf_tensor("c_qmask8", [E, E], F32).ap()
        wnT = [nc.alloc_sbuf_tensor(f"c_wnT{c}", [P, D], BF16).ap() for c in range(nd)]
        wsT = [nc.alloc_sbuf_tensor(f"c_wsT{c}", [P, D], BF16).ap() for c in range(nd)]
        scat = [
            nc.alloc_sbuf_tensor(f"c_scat{i}", [P, 2 * P], F32).ap()
            for i in range(NSC)
        ]
        # per chunk: col0=b, col1=1, col2=bmsg, col32=1, col33=a, col34=1
        # (groups at 0 and 32 so the transposed rows start at legal base
        # partitions for matmul operands)
        ab6_s = [
            nc.alloc_sbuf_tensor(f"c_ab6_{i}", [P, 35], BF16).ap() for i in range(NSC)
        ]

        make_identity(nc, I128)
        nc.gpsimd.tensor_copy(I128b, I128)
        nc.gpsimd.memset(ones_row, 1.0)
        nc.gpsimd.tensor_copy(ones_row_b, ones_row)
        for t_ in ab6_s:
            nc.gpsimd.memset(t_[:, 0:35], 0.0)
            nc.gpsimd.memset(t_[:, 1:2], 1.0)
            nc.gpsimd.memset(t_[:, 32:33], 1.0)
            nc.gpsimd.memset(t_[:, 34:35], 1.0)
        # qmask8: 0.25 off-diagonal, 0 on diagonal
        nc.gpsimd.memset(qmask8, 0.25)
        nc.gpsimd.affine_select(
            out=qmask8,
            in_=qmask8,
            compare_op=OP.not_equal,
            fill=0.0,
            base=0,
            pattern=[[-1, E]],
            channel_multiplier=1,
        )
        nc.gpsimd.memset(hadj_blk, 0.0)
        for s in scat:
            nc.gpsimd.tensor_copy(s[:, P : 2 * P].bitcast(F32R), I128)

        # ---- setup: params -> transposed/broadcast SBUF form ----
        with (
            tc.tile_pool(name="su", bufs=2) as su,
            tc.tile_pool(name="su_ps", bufs=2, space="PSUM") as sups,
        ):
            for mdram, dst in ((wn_dram, wnT), (ws_dram, wsT)):
                for oc in range(nd):
                    nat = su.tile([P, D], F32, tag="wnat")
                    nc.sync.dma_start(nat[:], mdram[oc * P : (oc + 1) * P, :])
                    for dc in range(nd):
                        ps = sups.tile([P, P], F32, tag="tps")
                        nc.tensor.transpose(
                            ps[:], nat[:, dc * P : (dc + 1) * P], I128
                        )
                        nc.vector.tensor_copy(dst[dc][:, oc * P : (oc + 1) * P], ps[:])

            wmsg_sb = su.tile([1, 2 * D], F32)
            nc.sync.dma_start(wmsg_sb[:], wmsg_dram[:])
            psa = sups.tile([P, D], F32, tag="bps")
            nc.tensor.matmul(psa[:], lhsT=ones_row, rhs=wmsg_sb[:, 0:D])
            nc.vector.tensor_copy(wa_rep, psa[:])
            psb = sups.tile([P, D], F32, tag="bps")
            nc.tensor.matmul(psb[:], lhsT=ones_row, rhs=wmsg_sb[:, D : 2 * D])
            nc.vector.tensor_copy(wb_rep, psb[:])

            bmsg_sb = su.tile([1, 1], F32)
            nc.sync.dma_start(bmsg_sb[:], bmsg_dram[:])
            psm = sups.tile([P, 1], F32, tag="sps")
            nc.tensor.matmul(psm[:], lhsT=ones_row, rhs=bmsg_sb[:])
            for t_ in ab6_s:
                nc.vector.tensor_copy(t_[:, 2:3], psm[:])

            bn_sb = su.tile([1, D], F32)
            bs_sb = su.tile([1, D], F32)
            nc.sync.dma_start(bn_sb[:], bn_dram[:])
            nc.sync.dma_start(bs_sb[:], bs_dram[:])
            nc.vector.tensor_add(bias_row_b, bn_sb[:], bs_sb[:])

            # adjacency: need adj^T blocks. Load natural, PE-transpose 8x8.
            adjn = su.tile([E, E], F32)
            nc.sync.dma_start(adjn[:], adj_dram[:])
            psd = sups.tile([E, E], F32, tag="sps")
            nc.tensor.transpose(psd[:], adjn[:], I128[0:E, 0:E])
            adjT = su.tile([E, E], F32)
            nc.vector.tensor_copy(adjT[:], psd[:])
            t8 = su.tile([E, E], F32)
            nc.scalar.activation(t8[:], adjT[:], AF.Tanh, scale=0.5)
            h8 = su.tile([E, E], BF16)
            # 0.25*(tanh+1) masked = 0.5*sigmoid(adjL^T) off-diagonal
            nc.vector.scalar_tensor_tensor(
                out=h8[:], in0=t8[:], scalar=1.0, in1=qmask8,
                op0=OP.add, op1=OP.mult,
            )
            # DMA (not DVE): block starts are not 32-aligned partitions
            for t in range(P // E):
                nc.sync.dma_start(
                    hadj_blk[t * E : (t + 1) * E, t * E : (t + 1) * E], h8[:]
                )

        # ---- main loop (software-pipelined, 6-stage skew) ----
        with (
            tc.tile_pool(name="xp", bufs=7) as xp,
            tc.tile_pool(name="sc", bufs=3) as scp,
            tc.tile_pool(name="small", bufs=4) as smp,
            tc.tile_pool(name="mid", bufs=3) as midp,
            tc.tile_pool(name="ztp", bufs=3) as ztp,
            tc.tile_pool(name="op", bufs=4) as op_,
            tc.tile_pool(name="ps_t", bufs=1, space="PSUM") as ps_t,
            tc.tile_pool(name="ps_o", bufs=2, space="PSUM") as ps_o,
            tc.tile_pool(name="ps_c", bufs=1, space="PSUM") as ps_c,
            tc.tile_pool(name="ps_b", bufs=3, space="PSUM") as ps_b,
        ):
            xins = {}
            lhsBs = {}
            rhsAs = {}
            bigs = {}
            zts = {}

            def stage_a(c):
                rows = slice(c * P, (c + 1) * P)
                xin = xp.tile([P, D], F32, tag="xin")
                nc.sync.dma_start(xin[:].bitcast(F32R), x_dram[rows, :].bitcast(F32R))
                xins[c] = xin

            def stage_c(c):
                ab6 = ab6_s[c % NSC]
                xin = xins[c]
                scr = scp.tile([P, D], BF16, tag="scr")
                nc.vector.scalar_tensor_tensor(
                    out=scr[:], in0=xin[:], scalar=0.0, in1=wb_rep,
                    op0=OP.bypass, op1=OP.mult, accum_out=ab6[:, 0:1],
                )
                scr2 = scp.tile([P, D], BF16, tag="scr")
                nc.vector.scalar_tensor_tensor(
                    out=scr2[:], in0=xin[:], scalar=0.0, in1=wa_rep,
                    op0=OP.bypass, op1=OP.mult, accum_out=ab6[:, 33:34],
                )

            def stage_d(c):
                abT = ps_t.tile([35, P], BF16, tag="abT")
                nc.tensor.transpose(abT[:], ab6_s[c % NSC][:], I128b)
                lhsB = smp.tile([3, P], BF16, tag="lhsB")
                rhsA = smp.tile([3, P], BF16, tag="rhsA")
                nc.vector.tensor_copy(lhsB[:], abT[0:3, :])
                nc.vector.tensor_copy(rhsA[:], abT[32:35, :])
                lhsBs[c] = lhsB
                rhsAs[c] = rhsA

            def stage_e(c):
                outer = ps_o.tile([P, P], F32, tag="outer")
                nc.tensor.matmul(outer[:], lhsT=lhsBs.pop(c)[:], rhs=rhsAs.pop(c)[:])
                th = midp.tile([P, P], BF16, tag="th")
                nc.scalar.activation(th[:], outer[:], AF.Tanh, scale=0.5)
                th1 = midp.tile([P, P], BF16, tag="th1")
                nc.gpsimd.tensor_scalar_add(th1[:], th[:], 1.0)
                sb = scat[c % NSC]
                nc.gpsimd.tensor_mul(sb[:, 0:P].bitcast(F32R), th1[:], hadj_blk)
                big = ps_b.tile([P, D], F32, tag="big")
                bigs[c] = big

            def stage_f1(c):
                xin = xins.pop(c)
                sb = scat[c % NSC]
                cmb = ps_c.tile([P, 2 * P * nd], F32, tag="cmb")
                for dc in range(nd):
                    nc.tensor.matmul(
                        cmb[:, 2 * P * dc : 2 * P * (dc + 1)],
                        lhsT=xin[:, dc * P : (dc + 1) * P].bitcast(F32R),
                        rhs=sb[:].bitcast(F32R),
                    )
                zt = ztp.tile([P, 2 * P * nd], BF16, tag="zt")
                nc.scalar.copy(zt[:], cmb[:])
                zts[c] = zt

            def stage_f2(c):
                rows = slice(c * P, (c + 1) * P)
                zt = zts.pop(c)
                big = bigs.pop(c)
                nc.tensor.matmul(
                    big[:], lhsT=ones_row_b, rhs=bias_row_b,
                    start=True, stop=False,
                )
                for dc in range(nd):
                    nc.tensor.matmul(
                        big[:],
                        lhsT=zt[:, 2 * P * dc : 2 * P * dc + P],
                        rhs=wnT[dc],
                        start=False,
                        stop=False,
                    )
                for dc in range(nd):
                    nc.tensor.matmul(
                        big[:],
                        lhsT=zt[:, 2 * P * dc + P : 2 * P * (dc + 1)],
                        rhs=wsT[dc],
                        start=False,
                        stop=(dc == nd - 1),
                    )
                osb = op_.tile([P, D], F32, tag="osb")
                nc.scalar.activation(
                    osb[:], big[:],
                    AF.Tanh if gelu_tanh_standin else AF.Gelu, scale=1.0,
                )
                nc.sync.dma_start(out_dram[rows, :], osb[:])

            stages = [stage_a, stage_c, stage_d, stage_e, stage_f1, stage_f2]
            for i in range(n_chunks + len(stages) - 1):
                for s_idx, fn in enumerate(stages):
                    c = i - s_idx
                    if 0 <= c < n_chunks:
                        fn(c)

    nc.compile()
    return nc


_CACHE = {}


def _get_nc():
    if "nc" not in _CACHE:
        _CACHE["nc"] = build_nc()
    return _CACHE["nc"]


def _make_in_maps(inputs):
    x = np.ascontiguousarray(np.asarray(inputs["expert_features"], np.float32))
    assert x.shape == (B, L, E, D)
    shards = x.reshape(N_CORES, ROWS_PER_CORE, D)
    params = {
        "W_neighbor": np.ascontiguousarray(np.asarray(inputs["W_neighbor"], np.float32)),
        "b_neighbor": np.asarray(inputs["b_neighbor"], np.float32).reshape(1, D),
        "W_self": np.ascontiguousarray(np.asarray(inputs["W_self"], np.float32)),
        "b_self": np.asarray(inputs["b_self"], np.float32).reshape(1, D),
        "w_msg": np.asarray(inputs["w_msg"], np.float32).reshape(1, 2 * D),
        "b_msg": np.asarray(inputs["b_msg"], np.float32).reshape(1, 1),
        "adjacency_logits": np.ascontiguousarray(
            np.asarray(inputs["adjacency_logits"], np.float32)
        ),
    }
    return [dict(expert_features=shards[c], **params) for c in range(N_CORES)]


def _run(inputs, trace=False):
    nc = _get_nc()
    in_maps = _make_in_maps(inputs)
    res = bass_utils.run_bass_kernel_spmd(
        nc, in_maps, core_ids=list(range(N_CORES)), trace=trace
    )
    out = np.stack([res.results[c]["out"] for c in range(N_CORES)], axis=0)
    return out.reshape(B, L, E, D), res


def kernel(**inputs):
    out, _ = _run(inputs, trace=False)
    return out


# revision 21
# speedup vs baseline: 1.2907x; 1.2907x over previous
"""ExpertGraphConv Trainium2 kernel (bf16 pipeline, native gelu, 6-stage skew).

Full inputs in, full output out. Shards batch dim (B=8) across 8 NeuronCores;
params replicated. Each core processes 2048 tokens x 8 experts = 16384 rows.

Math per (token t, expert i):
  adj = sigmoid(adjacency_logits); wa, wb = w_msg[:D], w_msg[D:]
  a[t,i] = x[t,i] . wa ; b[t,j] = x[t,j] . wb
  strength[t,i,j] = adj[i,j] * sigmoid(a[t,i]+b[t,j]+b_msg) * (i != j)
  msg[t,i] = sum_j strength[t,i,j] x[t,j]
  out = gelu(msg @ Wn^T + x @ Ws^T + bn + bs)

Per 128-row chunk (16 tokens x 8 experts), software-pipelined so every PE
instruction's cross-engine inputs are produced >= 1 full stage earlier:
  A(c):  DMA xin (fp32)
  B(c):  gpsimd cast -> xbf (bf16)
  C(c):  DVE a/b row-sums (fused mult+accum) -> ab6 rotating tile
  D(c):  PE transpose of ab6 ([128,35] bf16, groups at cols 0/32 so the
         transposed operands sit at legal base partitions) + DVE copies
  E(c):  PE K=3 outer matmul (scores for all 64 expert pairs) + ACT tanh
         + DVE strength stt + DVE bias prewrite into big PSUM
  F1(c): PE cmb: one bf16 matmul per d-chunk, rhs=[Sblk | I128] gives
         msg^T and x^T together in PSUM; ACT copy -> bf16 zt
  F2(c): 8 accumulating bf16 matmuls (start=False onto the prewritten
         bias) + native Gelu from PSUM + DMA out
Emission per iteration i: A(i), B(i-1), C(i-2), D(i-3), E(i-4), F1(i-5),
F2(i-6). The gelu/tanh pair lives in the 'gelu_and_others' ACT table set:
no table reloads in steady state.
"""

import math
from contextlib import ExitStack

import numpy as np

import concourse.bacc as bacc
import concourse.mybir as mybir
import concourse.tile as tile
from concourse import bass_utils
from concourse.masks import make_identity

F32 = mybir.dt.float32
F32R = mybir.dt.float32r
BF16 = mybir.dt.bfloat16
AF = mybir.ActivationFunctionType
OP = mybir.AluOpType

B, L, E, D = 8, 2048, 8, 512
N_CORES = 8
P = 128
ROWS_PER_CORE = (B // N_CORES) * L * E  # 16384
NSC = 8  # rotation depth of the static rotating buffers


def build_nc(n_rows=ROWS_PER_CORE, gelu_tanh_standin=False):
    assert n_rows % P == 0
    n_chunks = n_rows // P
    nd = D // P  # 4 d-chunks

    nc = bacc.Bacc(
        "TRN2", target_bir_lowering=False, debug=False, num_devices=N_CORES
    )

    x_dram = nc.dram_tensor("expert_features", [n_rows, D], F32, kind="ExternalInput").ap()
    wn_dram = nc.dram_tensor("W_neighbor", [D, D], F32, kind="ExternalInput").ap()
    bn_dram = nc.dram_tensor("b_neighbor", [1, D], F32, kind="ExternalInput").ap()
    ws_dram = nc.dram_tensor("W_self", [D, D], F32, kind="ExternalInput").ap()
    bs_dram = nc.dram_tensor("b_self", [1, D], F32, kind="ExternalInput").ap()
    wmsg_dram = nc.dram_tensor("w_msg", [1, 2 * D], F32, kind="ExternalInput").ap()
    bmsg_dram = nc.dram_tensor("b_msg", [1, 1], F32, kind="ExternalInput").ap()
    adj_dram = nc.dram_tensor("adjacency_logits", [E, E], F32, kind="ExternalInput").ap()
    out_dram = nc.dram_tensor("out", [n_rows, D], F32, kind="ExternalOutput").ap()

    with tile.TileContext(nc) as tc, ExitStack() as ctx:
        # ---- static SBUF tensors ----
        I128 = nc.alloc_sbuf_tensor("c_I128", [P, P], F32).ap()
        I128b = nc.alloc_sbuf_tensor("c_I128b", [P, P], BF16).ap()
        wa_rep = nc.alloc_sbuf_tensor("c_wa_rep", [P, D], BF16).ap()
        wb_rep = nc.alloc_sbuf_tensor("c_wb_rep", [P, D], BF16).ap()
        hadj_blk = nc.alloc_sbuf_tensor("c_hadj_blk", [P, P], BF16).ap()
        ones_row = nc.alloc_sbuf_tensor("c_ones_row", [1, P], F32).ap()
        ones_row_b = nc.alloc_sbuf_tensor("c_ones_row_b", [1, P], BF16).ap()
        bias_row_b = nc.alloc_sbuf_tensor("c_bias_row_b", [1, D], BF16).ap()
        qmask8 = nc.alloc_sbuf_tensor("c_qmask8", [E, E], F32).ap()
        wnT = [nc.alloc_sbuf_tensor(f"c_wnT{c}", [P, D], BF16).ap() for c in range(nd)]
        wsT = [nc.alloc_sbuf_tensor(f"c_wsT{c}", [P, D], BF16).ap() for c in range(nd)]
        scat = [
            nc.alloc_sbuf_tensor(f"c_scat{i}", [P, 2 * P], BF16).ap()
            for i in range(NSC)
        ]
        # per chunk: col0=b, col1=1, col2=bmsg, col32=1, col33=a, col34=1
        # (groups at 0 and 32 so the transposed rows start at legal base
        # partitions for matmul operands)
        ab6_s = [
            nc.alloc_sbuf_tensor(f"c_ab6_{i}", [P, 35], BF16).ap() for i in range(NSC)
        ]

        make_identity(nc, I128)
        nc.gpsimd.tensor_copy(I128b, I128)
        nc.gpsimd.memset(ones_row, 1.0)
        nc.gpsimd.tensor_copy(ones_row_b, ones_row)
        for t_ in ab6_s:
            nc.gpsimd.memset(t_[:, 0:35], 0.0)
            nc.gpsimd.memset(t_[:, 1:2], 1.0)
            nc.gpsimd.memset(t_[:, 32:33], 1.0)
            nc.gpsimd.memset(t_[:, 34:35], 1.0)
        # qmask8: 0.25 off-diagonal, 0 on diagonal
        nc.gpsimd.memset(qmask8, 0.25)
        nc.gpsimd.affine_select(
            out=qmask8,
            in_=qmask8,
            compare_op=OP.not_equal,
            fill=0.0,
            base=0,
            pattern=[[-1, E]],
            channel_multiplier=1,
        )
        nc.gpsimd.memset(hadj_blk, 0.0)
        for s in scat:
            nc.gpsimd.tensor_copy(s[:, P : 2 * P], I128b)

        # ---- setup: params -> transposed/broadcast SBUF form ----
        with (
            tc.tile_pool(name="su", bufs=2) as su,
            tc.tile_pool(name="su_ps", bufs=2, space="PSUM") as sups,
        ):
            for mdram, dst in ((wn_dram, wnT), (ws_dram, wsT)):
                for oc in range(nd):
                    nat = su.tile([P, D], F32, tag="wnat")
                    nc.sync.dma_start(nat[:], mdram[oc * P : (oc + 1) * P, :])
                    for dc in range(nd):
                        ps = sups.tile([P, P], F32, tag="tps")
                        nc.tensor.transpose(
                            ps[:], nat[:, dc * P : (dc + 1) * P], I128
                        )
                        nc.vector.tensor_copy(dst[dc][:, oc * P : (oc + 1) * P], ps[:])

            wmsg_sb = su.tile([1, 2 * D], F32)
            nc.sync.dma_start(wmsg_sb[:], wmsg_dram[:])
            psa = sups.tile([P, D], F32, tag="bps")
            nc.tensor.matmul(psa[:], lhsT=ones_row, rhs=wmsg_sb[:, 0:D])
            nc.vector.tensor_copy(wa_rep, psa[:])
            psb = sups.tile([P, D], F32, tag="bps")
            nc.tensor.matmul(psb[:], lhsT=ones_row, rhs=wmsg_sb[:, D : 2 * D])
            nc.vector.tensor_copy(wb_rep, psb[:])

            bmsg_sb = su.tile([1, 1], F32)
            nc.sync.dma_start(bmsg_sb[:], bmsg_dram[:])
            psm = sups.tile([P, 1], F32, tag="sps")
            nc.tensor.matmul(psm[:], lhsT=ones_row, rhs=bmsg_sb[:])
            for t_ in ab6_s:
                nc.vector.tensor_copy(t_[:, 2:3], psm[:])

            bn_sb = su.tile([1, D], F32)
            bs_sb = su.tile([1, D], F32)
            nc.sync.dma_start(bn_sb[:], bn_dram[:])
            nc.sync.dma_start(bs_sb[:], bs_dram[:])
            nc.vector.tensor_add(bias_row_b, bn_sb[:], bs_sb[:])

            # adjacency: need adj^T blocks. Load natural, PE-transpose 8x8.
            adjn = su.tile([E, E], F32)
            nc.sync.dma_start(adjn[:], adj_dram[:])
            psd = sups.tile([E, E], F32, tag="sps")
            nc.tensor.transpose(psd[:], adjn[:], I128[0:E, 0:E])
            adjT = su.tile([E, E], F32)
            nc.vector.tensor_copy(adjT[:], psd[:])
            t8 = su.tile([E, E], F32)
            nc.scalar.activation(t8[:], adjT[:], AF.Tanh, scale=0.5)
            h8 = su.tile([E, E], BF16)
            # 0.25*(tanh+1) masked = 0.5*sigmoid(adjL^T) off-diagonal
            nc.vector.scalar_tensor_tensor(
                out=h8[:], in0=t8[:], scalar=1.0, in1=qmask8,
                op0=OP.add, op1=OP.mult,
            )
            # DMA (not DVE): block starts are not 32-aligned partitions
            for t in range(P // E):
                nc.sync.dma_start(
                    hadj_blk[t * E : (t + 1) * E, t * E : (t + 1) * E], h8[:]
                )

        # ---- main loop (software-pipelined, 6-stage skew) ----
        with (
            tc.tile_pool(name="xp", bufs=4) as xp,
            tc.tile_pool(name="xbp", bufs=7) as xbp,
            tc.tile_pool(name="sc", bufs=3) as scp,
            tc.tile_pool(name="small", bufs=4) as smp,
            tc.tile_pool(name="mid", bufs=3) as midp,
            tc.tile_pool(name="ztp", bufs=3) as ztp,
            tc.tile_pool(name="op", bufs=4) as op_,
            tc.tile_pool(name="ps_t", bufs=1, space="PSUM") as ps_t,
            tc.tile_pool(name="ps_o", bufs=2, space="PSUM") as ps_o,
            tc.tile_pool(name="ps_c", bufs=1, space="PSUM") as ps_c,
            tc.tile_pool(name="ps_b", bufs=3, space="PSUM") as ps_b,
        ):
            xins = {}
            xbfs = {}
            lhsBs = {}
            rhsAs = {}
            bigs = {}
            zts = {}

            def stage_a(c):
                rows = slice(c * P, (c + 1) * P)
                xin = xp.tile([P, D], F32, tag="xin")
                nc.sync.dma_start(xin[:], x_dram[rows, :])
                xins[c] = xin

            def stage_b(c):
                xbf = xbp.tile([P, D], BF16, tag="xbf")
                nc.gpsimd.tensor_copy(xbf[:], xins.pop(c)[:])
                xbfs[c] = xbf

            def stage_c(c):
                ab6 = ab6_s[c % NSC]
                xbf = xbfs[c]
                scr = scp.tile([P, D], BF16, tag="scr")
                nc.vector.scalar_tensor_tensor(
                    out=scr[:], in0=xbf[:], scalar=0.0, in1=wb_rep,
                    op0=OP.bypass, op1=OP.mult, accum_out=ab6[:, 0:1],
                )
                scr2 = scp.tile([P, D], BF16, tag="scr")
                nc.vector.scalar_tensor_tensor(
                    out=scr2[:], in0=xbf[:], scalar=0.0, in1=wa_rep,
                    op0=OP.bypass, op1=OP.mult, accum_out=ab6[:, 33:34],
                )

            def stage_d(c):
                abT = ps_t.tile([35, P], BF16, tag="abT")
                nc.tensor.transpose(abT[:], ab6_s[c % NSC][:], I128b)
                lhsB = smp.tile([3, P], BF16, tag="lhsB")
                rhsA = smp.tile([3, P], BF16, tag="rhsA")
                nc.vector.tensor_copy(lhsB[:], abT[0:3, :])
                nc.vector.tensor_copy(rhsA[:], abT[32:35, :])
                lhsBs[c] = lhsB
                rhsAs[c] = rhsA

            def stage_e(c):
                outer = ps_o.tile([P, P], F32, tag="outer")
                nc.tensor.matmul(outer[:], lhsT=lhsBs.pop(c)[:], rhs=rhsAs.pop(c)[:])
                th = midp.tile([P, P], BF16, tag="th")
                nc.scalar.activation(th[:], outer[:], AF.Tanh, scale=0.5)
                sb = scat[c % NSC]
                nc.vector.scalar_tensor_tensor(
                    out=sb[:, 0:P], in0=th[:], scalar=1.0, in1=hadj_blk,
                    op0=OP.add, op1=OP.mult,
                )
                big = ps_b.tile([P, D], F32, tag="big")
                bigs[c] = big

            def stage_f1(c):
                xbf = xbfs.pop(c)
                sb = scat[c % NSC]
                cmb = ps_c.tile([P, 2 * P * nd], F32, tag="cmb")
                for dc in range(nd):
                    nc.tensor.matmul(
                        cmb[:, 2 * P * dc : 2 * P * (dc + 1)],
                        lhsT=xbf[:, dc * P : (dc + 1) * P],
                        rhs=sb[:],
                    )
                zt = ztp.tile([P, 2 * P * nd], BF16, tag="zt")
                nc.scalar.copy(zt[:], cmb[:])
                zts[c] = zt

            def stage_f2(c):
                rows = slice(c * P, (c + 1) * P)
                zt = zts.pop(c)
                big = bigs.pop(c)
                nc.tensor.matmul(
                    big[:], lhsT=ones_row_b, rhs=bias_row_b,
                    start=True, stop=False,
                )
                for dc in range(nd):
                    nc.tensor.matmul(
                        big[:],
                        lhsT=zt[:, 2 * P * dc : 2 * P * dc + P],
                        rhs=wnT[dc],
                        start=False,
                        stop=False,
                    )
                for dc in range(nd):
                    nc.tensor.matmul(
                        big[:],
                        lhsT=zt[:, 2 * P * dc + P : 2 * P * (dc + 1)],
                        rhs=wsT[dc],
                        start=False,
                        stop=(dc == nd - 1),
                    )
                osb = op_.tile([P, D], F32, tag="osb")
                nc.scalar.activation(
                    osb[:], big[:],
                    AF.Tanh if gelu_tanh_standin else AF.Gelu, scale=1.0,
                )
                nc.sync.dma_start(out_dram[rows, :], osb[:])

            stage_offsets = [
                (stage_a, 0), (stage_e, 4), (stage_d, 3), (stage_b, 1),
                (stage_c, 2), (stage_f1, 5), (stage_f2, 6),
            ]
            n_stages = 7
            for i in range(n_chunks + n_stages - 1):
                for fn, off in stage_offsets:
                    c = i - off
                    if 0 <= c < n_chunks:
                        fn(c)

    nc.compile()
    return nc


_CACHE = {}


def _get_nc():
    if "nc" not in _CACHE:
        _CACHE["nc"] = build_nc()
    return _CACHE["nc"]


def _make_in_maps(inputs):
    x = np.ascontiguousarray(np.asarray(inputs["expert_features"], np.float32))
    assert x.shape == (B, L, E, D)
    shards = x.reshape(N_CORES, ROWS_PER_CORE, D)
    params = {
        "W_neighbor": np.ascontiguousarray(np.asarray(inputs["W_neighbor"], np.float32)),
        "b_neighbor": np.asarray(inputs["b_neighbor"], np.float32).reshape(1, D),
        "W_self": np.ascontiguousarray(np.asarray(inputs["W_self"], np.float32)),
        "b_self": np.asarray(inputs["b_self"], np.float32).reshape(1, D),
        "w_msg": np.asarray(inputs["w_msg"], np.float32).reshape(1, 2 * D),
        "b_msg": np.asarray(inputs["b_msg"], np.float32).reshape(1, 1),
        "adjacency_logits": np.ascontiguousarray(
            np.asarray(inputs["adjacency_logits"], np.float32)
        ),
    }
    return [dict(expert_features=shards[c], **params) for c in range(N_CORES)]


def _run(inputs, trace=False):
    nc = _get_nc()
    in_maps = _make_in_maps(inputs)
    res = bass_utils.run_bass_kernel_spmd(
        nc, in_maps, core_ids=list(range(N_CORES)), trace=trace
    )
    out = np.stack([res.results[c]["out"] for c in range(N_CORES)], axis=0)
    return out.reshape(B, L, E, D), res


def kernel(**inputs):
    out, _ = _run(inputs, trace=False)
    return out
